# revision 3
# baseline (speedup 1.0000x reference)
"""Fused LoRA-QKV projection kernel for 8 Trainium2 NeuronCores.

Computes  out = x @ W.T + b + scaling * concat_k((x @ A[k].T) @ B[k].T)
with x:[4,2048,4096] f32, W:[12288,4096], b:[12288], A:[3,16,4096],
B:[3,4096,16]  ->  out:[4,2048,12288] f32.

Sharding (Megatron column-parallel): out_dim (12288) split across 8 cores,
each of the 3 q/k/v chunks evenly split; core c owns rows
{k*4096 + c*512 .. k*4096 + (c+1)*512} of W/b and rows {c*512..(c+1)*512}
of each B[k].  x and A are replicated; each core emits its [tokens,1536]
slice and the host interleaves slices into the full output.

Per-core design (bf16 operands, f32 PSUM accumulation):
  - LoRA is folded into the resident weights on-device during W-prep:
    wt_eff[d,f] = W[f,d] + scaling * sum_r A[k,r,d]*B[k,f,r].  W tiles are
    PE-transposed into PSUM f32 and a K=16 matmul (lhsT = A natural layout,
    rhs = scaled B.T) accumulates the LoRA outer product into the same PSUM
    tile before a single DVE eviction to bf16 SBUF.  The main loop is then
    a pure GEMM + bias.
  - x tiles load f32 via the ACT HWDGE ring (so they never queue behind the
    W-prep loads on the SP ring), cast f32->bf16 on the Scalar engine, and
    are PE-transposed 8-at-a-time into one [128,1024] bf16 PSUM bank with a
    single DVE eviction per group.
  - base GEMM: per 128-token block, 3 accumulation chains (N=512, one PSUM
    bank each, 4-deep pool) over 32 K-tiles; bias is added during the
    PSUM->SBUF eviction (DVE tensor_add vs a broadcast bias tile) and each
    512-feature slice DMAs out as soon as it is evicted.
"""

import numpy as np

import concourse.bass as bass
import concourse.mybir as mybir
from concourse import bacc
from concourse.masks import make_identity
from concourse.tile import TileContext

IN_DIM = 4096
OUT_DIM = 12288
R = 16
SCALING = 32.0 / R
N_CORES = 8
TOKENS = 4 * 2048
FEATS = OUT_DIM // N_CORES          # 1536 per core
N_SLICE = 512                       # psum tile free size (one bank of fp32)
F_SLICES = FEATS // N_SLICE         # 3 (== adapter count; slice f <-> adapter f)
D_TILES = IN_DIM // 128             # 32
BF = mybir.dt.bfloat16
F32 = mybir.dt.float32


def build_nc(tokens=TOKENS):
    t_blocks = tokens // 128
    nc = bacc.Bacc()
    x = nc.declare_dram_parameter("x", [tokens, IN_DIM], F32, isOutput=False)
    w = nc.declare_dram_parameter("w", [FEATS, IN_DIM], F32, isOutput=False)
    bvec = nc.declare_dram_parameter("bvec", [FEATS], F32, isOutput=False)
    amat = nc.declare_dram_parameter("amat", [3 * R, IN_DIM], F32, isOutput=False)
    bmat = nc.declare_dram_parameter("bmat", [FEATS, R], F32, isOutput=False)
    out = nc.declare_dram_parameter("out", [tokens, FEATS], F32, isOutput=True)

    with TileContext(nc) as tc:
        with (
            tc.tile_pool(name="const", bufs=1) as const,
            tc.tile_pool(name="bpsum", bufs=4, space="PSUM") as bpsum,
        ):
            ident = const.tile([128, 128], BF, name="ident")
            make_identity(nc, ident)
            identf = const.tile([128, 128], F32, name="identf")
            make_identity(nc, identf)
            # W_eff.T resident: wt[:, j*FEATS + f] = W_eff[f, j*128 + p]
            wt = const.tile([128, D_TILES * FEATS], BF, name="wt")
            # A natural (bf16), adapter k at partitions 32k..32k+R (quadrant
            # bases 0/32/64 are the only legal matmul base partitions):
            # a_nat[32k + r, d]
            a_nat = const.tile([64 + R, IN_DIM], BF, name="a_nat")
            # B.T (bf16, pre-scaled by SCALING): btt[32k + r, floc]
            btt = const.tile([64 + R, FEATS], BF, name="btt")
            # bias broadcast across partitions
            bb = const.tile([128, FEATS], F32, name="bb")
            bap = bvec[:]
            bias_bcast = bass.AP(
                tensor=bap.tensor, offset=bap.offset,
                ap=[[0, 128]] + [list(d) for d in bap.ap],
            )
            nc.sync.dma_start(out=bb, in_=bias_bcast)

            with (
                tc.tile_pool(name="stage", bufs=2) as stage,
                tc.tile_pool(name="ppsum", bufs=2, space="PSUM") as tpsum,
            ):
                # ---- A prep: DMA each adapter to partitions 32k..32k+R of a
                #      padded f32 staging tile, then one base-aligned cast ----
                ast = stage.tile([64 + R, IN_DIM], F32, name="ast")
                for k in range(3):
                    nc.sync.dma_start(
                        out=ast[32 * k:32 * k + R, :],
                        in_=amat[R * k:R * (k + 1), :],
                    )
                nc.scalar.copy(a_nat, ast)
                # ---- B prep: load f32 natural [FEATS, R], cast+scale,
                #      PE-transpose 128-row chunks into btt[16k+r, floc] ----
                for c in range(FEATS // 128):
                    k = c // (N_SLICE // 128)      # adapter for this chunk
                    bst = stage.tile([128, R], F32, name="bst")
                    nc.sync.dma_start(out=bst, in_=bmat[c * 128:(c + 1) * 128, :])
                    bsc = stage.tile([128, R], BF, name="bsc")
                    nc.scalar.mul(bsc, bst, SCALING)
                    tpb = tpsum.tile([64 + R, 128], BF, name="tpb", tag="tpb")
                    nc.tensor.matmul(
                        tpb[32 * k:32 * k + R, :], bsc, ident,
                        is_transpose=True, tile_position=(0, 32 * k),
                    )
                    nc.vector.tensor_copy(
                        btt[32 * k:32 * k + R, c * 128:(c + 1) * 128],
                        tpb[32 * k:32 * k + R, :],
                    )
                # ---- W prep: load natural rows, cast, PE-transpose into
                #      PSUM f32, accumulate LoRA fold, evict to wt ----
                for i in range(FEATS // 128):
                    k = i // (N_SLICE // 128)      # adapter for this chunk
                    wst = stage.tile([128, IN_DIM], F32, name="wst", bufs=3)
                    nc.sync.dma_start(out=wst, in_=w[i * 128:(i + 1) * 128, :])
                    # 4 d-tiles (transpose + LoRA fold each) share one
                    # [128,512] f32 psum tile; single DVE eviction per group.
                    for g in range(D_TILES // 4):
                        tp = tpsum.tile([128, 512], F32, name="tp", tag="tp")
                        for u in range(4):
                            j = 4 * g + u
                            sl = slice(u * 128, (u + 1) * 128)
                            nc.tensor.transpose(
                                tp[:, sl], wst[:, j * 128:(j + 1) * 128], identf
                            )
                            # += scaling*(B[k] @ A[k]).T for this (d-tile, chunk)
                            nc.tensor.matmul(
                                tp[:, sl],
                                a_nat[32 * k:32 * k + R, j * 128:(j + 1) * 128],
                                btt[32 * k:32 * k + R, i * 128:(i + 1) * 128],
                                start=False, stop=True,
                            )
                        # strided eviction: tp column u*128+c -> wt column
                        # (4g+u)*FEATS + i*128 + c
                        wt_view = wt[:, :].rearrange(
                            "p (j f) -> p j f", j=D_TILES
                        )
                        nc.vector.tensor_copy(
                            wt_view[:, 4 * g:4 * g + 4, i * 128:(i + 1) * 128],
                            tp,
                        )

            with (
                tc.tile_pool(name="xin", bufs=2) as xin_pool,
                tc.tile_pool(name="xbf", bufs=2) as xbf_pool,
                tc.tile_pool(name="xt", bufs=3) as xt_pool,
                tc.tile_pool(name="osb", bufs=2) as osb_pool,
                tc.tile_pool(name="xpsum", bufs=4, space="PSUM") as xpsum,
            ):
                for t in range(t_blocks):
                    xin = xin_pool.tile([128, IN_DIM], F32, name="xin")
                    # ACT HWDGE ring: runs concurrently with the W-prep loads
                    # queued on the SP (sync) ring.
                    nc.scalar.dma_start(out=xin, in_=x[t * 128:(t + 1) * 128, :])
                    xbf = xbf_pool.tile([128, IN_DIM], BF, name="xbf")
                    nc.scalar.copy(xbf, xin)
                    xt = xt_pool.tile([128, IN_DIM], BF, name="xt")
                    # 8 transposes pack one [128,1024] bf16 psum tile (one
                    # 2KB bank); one DVE eviction per group of 8.
                    for g in range(D_TILES // 8):
                        tpx = xpsum.tile([128, 1024], BF, name="tpx", tag="tpx")
                        for u in range(8):
                            j = 8 * g + u
                            nc.tensor.transpose(
                                tpx[:, u * 128:(u + 1) * 128],
                                xbf[:, j * 128:(j + 1) * 128], ident,
                            )
                        nc.vector.tensor_copy(
                            xt[:, g * 1024:(g + 1) * 1024], tpx
                        )
                    osb = osb_pool.tile([128, FEATS], F32, name="osb")
                    for f in range(F_SLICES):
                        bp = bpsum.tile([128, N_SLICE], F32, name="bp")
                        for j in range(D_TILES):
                            nc.tensor.matmul(
                                bp, xt[:, j * 128:(j + 1) * 128],
                                wt[:, j * FEATS + f * N_SLICE: j * FEATS + (f + 1) * N_SLICE],
                                start=(j == 0), stop=(j == D_TILES - 1),
                            )
                        nc.vector.tensor_add(
                            osb[:, f * N_SLICE:(f + 1) * N_SLICE], bp,
                            bb[:, f * N_SLICE:(f + 1) * N_SLICE],
                        )
                        nc.sync.dma_start(
                            out=out[t * 128:(t + 1) * 128,
                                    f * N_SLICE:(f + 1) * N_SLICE],
                            in_=osb[:, f * N_SLICE:(f + 1) * N_SLICE],
                        )
    nc.compile()
    return nc


def shard_inputs(inputs, tokens=TOKENS):
    """Full inputs -> per-core in_maps (column-parallel on out_dim)."""
    x = np.ascontiguousarray(np.asarray(inputs["x"], dtype=np.float32)).reshape(
        tokens, IN_DIM
    )
    W = np.asarray(inputs["W"], dtype=np.float32).reshape(3, OUT_DIM // 3, IN_DIM)
    b = np.asarray(inputs["b"], dtype=np.float32).reshape(3, OUT_DIM // 3)
    A = np.asarray(inputs["A"], dtype=np.float32).reshape(3 * R, IN_DIM)
    B = np.asarray(inputs["B"], dtype=np.float32)  # [3, 4096, 16]
    in_maps = []
    for c in range(N_CORES):
        sl = slice(c * N_SLICE, (c + 1) * N_SLICE)
        in_maps.append({
            "x": x,
            "w": np.ascontiguousarray(W[:, sl, :]).reshape(FEATS, IN_DIM),
            "bvec": np.ascontiguousarray(b[:, sl]).reshape(FEATS),
            "amat": np.ascontiguousarray(A),
            "bmat": np.ascontiguousarray(B[:, sl, :]).reshape(FEATS, R),
        })
    return in_maps


def unshard_output(results, tokens=TOKENS):
    """Per-core [tokens, 1536] slices -> full [4, 2048, 12288]."""
    full = np.empty((tokens, 3, N_CORES, N_SLICE), dtype=np.float32)
    for c, res in enumerate(results):
        full[:, :, c, :] = res["out"].reshape(tokens, 3, N_SLICE)
    return full.reshape(4, 2048, OUT_DIM)


def run(inputs, tokens=TOKENS, **kwargs):
    from concourse.bass_utils import run_bass_kernel_spmd

    nc = build_nc(tokens)
    in_maps = shard_inputs(inputs, tokens)
    res = run_bass_kernel_spmd(
        nc, in_maps, core_ids=list(range(N_CORES)), **kwargs
    )
    return unshard_output(res.results, tokens), res


def kernel(**inputs) -> np.ndarray:
    out, _ = run(inputs)
    return out
